# revision 1
# baseline (speedup 1.0000x reference)
"""BERT self-attention (B=4, S=2048, D=1024, H=16) on 8 trn2 NeuronCores.

Sharding: core c -> (batch b = c//2, head-group hg = c%2, 8 heads each).
Each core computes out[b, :, hg*512:(hg+1)*512] independently; host
gathers. Inputs are pre-transposed on host so the contraction dim (d)
lands on SBUF partitions: xt = X.T [D,S], w{q,k,v}t = W.T shard [D,512].

On-device algorithm per core (all matmuls fp32r):
  Q^T, K^T: [o, s] pair-tiles (2 heads / 128 partitions), V: [s, o]
  augmented with a ones column per head (V_aug [s, h, 65]).
  Scores transposed per head: S^T[j, i] = K_h^T.T @ Q_h^T, head pairs
  row-packed on the two PE-array halves (K=64 each).
  U = exp(0.125*S^T + mask[j]) on ACT (mask = per-partition bias).
  ctx_u^T[dh+1, i] = V_aug.T @ U accumulated over j-tiles in PSUM; row 64
  is the softmax denominator (free rowsum via the ones column).
  Final: PE-transpose [65,128] chunks -> [128,65], DVE reciprocal of
  col 64 and tensor_scalar_mul -> out[s, o] tiles -> DMA.
"""

import numpy as np

import concourse.bass as bass
import concourse.tile as tile
from concourse import bacc, mybir
from concourse.bass_utils import run_bass_kernel_spmd
from concourse.masks import make_identity

B, S, D, H = 4, 2048, 1024, 16
DH = 64
O = 512  # per-core output width (8 heads)
HL = 8  # local heads per core
NP = 4  # head pairs per core
ST = S // 128  # 16 s-tiles
F32 = mybir.dt.float32
F32R = mybir.dt.float32r
EXP = mybir.ActivationFunctionType.Exp

_NC_CACHE = None


def build_nc():
    nc = bacc.Bacc(
        "TRN2",
        target_bir_lowering=False,
        debug=False,
        enable_asserts=True,
        num_devices=8,
    )
    xt = nc.dram_tensor("xt", [D, S], F32R, kind="ExternalInput").ap()
    wqt = nc.dram_tensor("wqt", [D, O], F32R, kind="ExternalInput").ap()
    wkt = nc.dram_tensor("wkt", [D, O], F32R, kind="ExternalInput").ap()
    wvt = nc.dram_tensor("wvt", [D, O], F32R, kind="ExternalInput").ap()
    bq = nc.dram_tensor("bq", [O], F32, kind="ExternalInput").ap()
    bk = nc.dram_tensor("bk", [O], F32, kind="ExternalInput").ap()
    bv = nc.dram_tensor("bv", [O], F32, kind="ExternalInput").ap()
    mask = nc.dram_tensor("mask", [S], F32, kind="ExternalInput").ap()
    out = nc.dram_tensor("out", [S, O], F32, kind="ExternalOutput").ap()

    with tile.TileContext(nc) as tc:
        _emit(nc, tc, xt, wqt, wkt, wvt, bq, bk, bv, mask, out)
    nc.compile()
    return nc


def _emit(nc, tc, xt, wqt, wkt, wvt, bq, bk, bv, mask, out):
    with (
        tc.tile_pool(name="singles", bufs=1) as singles,
        tc.tile_pool(name="persist", bufs=1) as persist,
        tc.tile_pool(name="psum", bufs=1, space="PSUM") as psum,
    ):
        ident = singles.tile([128, 128], F32)
        make_identity(nc, ident)
        mask_sb = singles.tile([128, ST], F32)
        nc.sync.dma_start(out=mask_sb, in_=mask.rearrange("(t p) -> p t", p=128))
        bq_sb = singles.tile([128, NP], F32)
        nc.sync.dma_start(out=bq_sb, in_=bq.rearrange("(t p) -> p t", p=128))
        bk_sb = singles.tile([128, NP], F32)
        nc.sync.dma_start(out=bk_sb, in_=bk.rearrange("(t p) -> p t", p=128))
        bv_bc = singles.tile([128, O], F32)
        nc.sync.dma_start(
            out=bv_bc, in_=bass.AP(tensor=bv.tensor, offset=0, ap=[[0, 128], [1, O]])
        )
        ones_sb = singles.tile([128, 1], F32)
        nc.vector.memset(ones_sb, 1.0)

        # persistent activations
        qts = [persist.tile([128, S], F32R, name=f"qt{p}", tag=f"qt{p}") for p in range(NP)]
        kts = [persist.tile([128, S], F32R, name=f"kt{p}", tag=f"kt{p}") for p in range(NP)]
        vaug = [
            persist.tile([128, HL, DH + 1], F32R, name=f"vaug{t}", tag=f"vaug{t}")
            for t in range(ST)
        ]

        stags = ("s0", "s1")

        with tc.tile_pool(name="proj", bufs=1) as proj:
            xts = []
            for dt in range(8):
                xti = proj.tile([128, S], F32R, name=f"xts{dt}", tag=f"xts{dt}")
                nc.sync.dma_start(out=xti, in_=xt[dt * 128 : (dt + 1) * 128, :])
                xts.append(xti)

            def load_w(wdram, label):
                wts = []
                for dt in range(8):
                    w = proj.tile([128, O], F32R, name=f"w{label}{dt}", tag="w", bufs=10)
                    nc.sync.dma_start(out=w, in_=wdram[dt * 128 : (dt + 1) * 128, :])
                    wts.append(w)
                return wts

            k = 0

            def qk_proj(wts, dsts, bias_sb, label):
                nonlocal k
                for p in range(NP):
                    for c in range(4):
                        ps = psum.tile(
                            [128, 512], F32, name=f"pp{label}{p}_{c}", tag=stags[k % 2]
                        )
                        k += 1
                        for dt in range(8):
                            nc.tensor.matmul(
                                ps,
                                wts[dt][:, p * 128 : (p + 1) * 128],
                                xts[dt][:, c * 512 : (c + 1) * 512],
                                start=(dt == 0),
                                stop=(dt == 7),
                            )
                        nc.vector.tensor_scalar_add(
                            dsts[p][:, c * 512 : (c + 1) * 512], ps, bias_sb[:, p : p + 1]
                        )

            wk_t = load_w(wkt, "k")
            qk_proj(wk_t, kts, bk_sb, "k")

            wv_t = load_w(wvt, "v")
            for st in range(ST):
                ps = psum.tile([128, O], F32, name=f"ppv{st}", tag=stags[k % 2])
                k += 1
                for dt in range(8):
                    nc.tensor.matmul(
                        ps,
                        xts[dt][:, st * 128 : (st + 1) * 128],
                        wv_t[dt],
                        start=(dt == 0),
                        stop=(dt == 7),
                    )
                va = vaug[st]
                for h in range(HL):
                    nc.vector.tensor_copy(out=va[:, h, DH : DH + 1], in_=ones_sb)
                for h in range(HL):
                    nc.vector.tensor_add(
                        va[:, h, 0:DH],
                        ps[:, h * DH : (h + 1) * DH],
                        bv_bc[:, h * DH : (h + 1) * DH],
                    )

            wq_t = load_w(wqt, "q")
            qk_proj(wq_t, qts, bq_sb, "q")

        with tc.tile_pool(name="attn", bufs=1) as attn:
            for p in range(NP):
                qtp, ktp = qts[p], kts[p]
                cxs = {
                    (ih, x): attn.tile(
                        [DH + 1, 1024], F32, name=f"cx{p}_{ih}_{x}", tag="cx", bufs=8
                    )
                    for ih in range(2)
                    for x in range(2)
                }
                for jt in range(ST):
                    for ih in range(2):
                        sps = []
                        for x in range(2):
                            sp = psum.tile(
                                [128, 1024], F32, name=f"s{p}_{ih}_{jt}_{x}",
                                tag=f"s{(2 * ih + x) % 4}"
                            )
                            sps.append(sp)
                        for c in range(2):
                            for x in range(2):
                                hp = slice(x * 64, x * 64 + 64)
                                ic = ih * 1024 + c * 512
                                nc.tensor.matmul(
                                    sps[x][:, c * 512 : (c + 1) * 512],
                                    ktp[hp, jt * 128 : (jt + 1) * 128],
                                    qtp[hp, ic : ic + 512],
                                    start=True,
                                    stop=True,
                                )
                        for x in range(2):
                            u = attn.tile(
                                [128, 1024], F32R, name=f"u{p}_{ih}_{jt}_{x}",
                                tag=f"u{x}", bufs=4
                            )
                            nc.scalar.activation(
                                u, sps[x], EXP, bias=mask_sb[:, jt : jt + 1], scale=0.125
                            )
                            pv = psum.tile(
                                [DH + 1, 1024], F32, name=f"pv{p}_{ih}_{jt}_{x}",
                                tag=f"s{(2 * ih + x) % 4}"
                            )
                            for c in range(2):
                                nc.tensor.matmul(
                                    pv[:, c * 512 : (c + 1) * 512],
                                    vaug[jt][:, 2 * p + x, :],
                                    u[:, c * 512 : (c + 1) * 512],
                                    start=True,
                                    stop=True,
                                )
                            if jt == 0:
                                nc.vector.tensor_copy(out=cxs[ih, x], in_=pv)
                            else:
                                nc.vector.tensor_add(cxs[ih, x], pv, cxs[ih, x])
                # drain: normalize + transpose + store (reads SBUF ctx directly)
                for ih in range(2):
                    for x in range(2):
                        hh = 2 * p + x
                        for it in range(8):
                            tp_ = psum.tile(
                                [128, DH + 1], F32, name=f"tp{p}_{ih}_{x}_{it}",
                                tag=f"s{it % 4}"
                            )
                            nc.tensor.transpose(
                                tp_, cxs[ih, x][:, it * 128 : (it + 1) * 128],
                                ident[0 : DH + 1, 0 : DH + 1]
                            )
                            rc = attn.tile([128, 1], F32, name=f"rc{p}_{ih}_{x}_{it}", tag="rc", bufs=6)
                            nc.vector.reciprocal(rc, tp_[:, DH : DH + 1])
                            ot = attn.tile([128, DH], F32, name=f"ot{p}_{ih}_{x}_{it}", tag="ot", bufs=6)
                            nc.vector.tensor_scalar_mul(ot, tp_[:, 0:DH], rc)
                            row = ih * 1024 + it * 128
                            nc.sync.dma_start(
                                out=out[row : row + 128, hh * DH : (hh + 1) * DH], in_=ot
                            )


def _make_in_maps(hidden_states, attention_mask, Wq, bq, Wk, bk, Wv, bv):
    in_maps = []
    for c in range(8):
        b, hg = divmod(c, 2)
        sl = slice(hg * O, (hg + 1) * O)
        in_maps.append(
            {
                "xt": np.ascontiguousarray(hidden_states[b].T),
                "wqt": np.ascontiguousarray(Wq[sl, :].T),
                "wkt": np.ascontiguousarray(Wk[sl, :].T),
                "wvt": np.ascontiguousarray(Wv[sl, :].T),
                "bq": np.ascontiguousarray(bq[sl]),
                "bk": np.ascontiguousarray(bk[sl]),
                "bv": np.ascontiguousarray(bv[sl]),
                "mask": np.ascontiguousarray(attention_mask[b, 0, 0, :]),
            }
        )
    return in_maps


def _gather(results):
    out = np.empty((B, S, D), dtype=np.float32)
    for c in range(8):
        b, hg = divmod(c, 2)
        out[b, :, hg * O : (hg + 1) * O] = results[c]["out"]
    return out


def kernel(hidden_states, attention_mask, Wq, bq, Wk, bk, Wv, bv, **run_kwargs):
    global _NC_CACHE
    args = [hidden_states, attention_mask, Wq, bq, Wk, bk, Wv, bv]
    args = [np.asarray(a, dtype=np.float32) for a in args]
    if _NC_CACHE is None:
        _NC_CACHE = build_nc()
    in_maps = _make_in_maps(*args)
    res = run_bass_kernel_spmd(_NC_CACHE, in_maps, core_ids=list(range(8)), **run_kwargs)
    kernel.last_result = res
    return _gather(res.results)



# revision 6
# speedup vs baseline: 1.6498x; 1.6498x over previous
"""BERT self-attention (B=4, S=2048, D=1024, H=16) on 8 trn2 NeuronCores.

Sharding: core c -> (batch b = c//2, head-group hg = c%2, 8 heads each).
Each core computes out[b, :, hg*512:(hg+1)*512]; host gathers. Inputs are
pre-transposed AND cast to bf16 on host so matmuls run at full PE rate
(fp32 feed is half rate): xt = X.T [D,S] bf16, w{q,k,v}t = W.T [D,512] bf16.

On-device per core, all matmul operands bf16, PSUM accumulation fp32:
  Q^T, K^T: [o, s] pair-tiles (2 heads / 128 partitions); V_aug [s, h, 65]
  with a leading ones column per head (row 0 of ctx psum = softmax denom).
  Scores S^T[j, i] per head pair packed into one [128, 2, 512] PSUM tile
  (the two heads' K=64 matmuls run concurrently in disjoint PE row groups).
  U = exp(0.125*S^T + mask[j]) in ONE activation instr over 1024 free
  elems -> bf16 SBUF.
  ctx_aug[1+dh, i] accumulated over all 16 j-tiles directly in PSUM.
  Drain: DVE reciprocal of denom row, GpSimd partition-broadcast, DVE
  multiply -> bf16 out tile. Output is stored TRANSPOSED [o, s] in DRAM;
  the host transposes for free during the gather.
  Q/K projections for pair p+1 are interleaved into pair p's attention
  loop (1 matmul per score-tile slot) to fill the ACT-bound PE slack.
"""

import numpy as np
import ml_dtypes

import concourse.bass as bass
import concourse.tile as tile
from concourse import bacc, mybir
from concourse.bass_utils import run_bass_kernel_spmd

B, S, D, H = 4, 2048, 1024, 16
DH = 64
O = 512  # per-core output width (8 heads)
HL = 8  # local heads per core
NP = 4  # head pairs per core
ST = S // 128  # 16 s-tiles
NIC = 4  # i-chunks of 512
F32 = mybir.dt.float32
BF16 = mybir.dt.bfloat16
EXP = mybir.ActivationFunctionType.Exp

_NC_CACHE = None


def build_nc():
    nc = bacc.Bacc(
        "TRN2",
        target_bir_lowering=False,
        debug=False,
        enable_asserts=True,
        num_devices=8,
    )
    xt = nc.dram_tensor("xt", [D, S], BF16, kind="ExternalInput").ap()
    wqt = nc.dram_tensor("wqt", [D, O], BF16, kind="ExternalInput").ap()
    wkt = nc.dram_tensor("wkt", [D, O], BF16, kind="ExternalInput").ap()
    wvt = nc.dram_tensor("wvt", [D, O], BF16, kind="ExternalInput").ap()
    bq = nc.dram_tensor("bq", [O], F32, kind="ExternalInput").ap()
    bk = nc.dram_tensor("bk", [O], F32, kind="ExternalInput").ap()
    bv = nc.dram_tensor("bv", [O], F32, kind="ExternalInput").ap()
    mask = nc.dram_tensor("mask", [S], F32, kind="ExternalInput").ap()
    # transposed output: [o, s]; host transposes during gather
    out = nc.dram_tensor("out", [O, S], BF16, kind="ExternalOutput").ap()

    with tile.TileContext(nc) as tc:
        _emit(nc, tc, xt, wqt, wkt, wvt, bq, bk, bv, mask, out)
    nc.compile()
    return nc


def _emit(nc, tc, xt, wqt, wkt, wvt, bq, bk, bv, mask, out):
    with (
        tc.tile_pool(name="singles", bufs=1) as singles,
        tc.tile_pool(name="persist", bufs=1) as persist,
        tc.tile_pool(name="work", bufs=1) as work,
        tc.tile_pool(name="psum", bufs=1, space="PSUM") as psum,
    ):
        mask_sb = singles.tile([128, ST], F32)
        nc.sync.dma_start(out=mask_sb, in_=mask.rearrange("(t p) -> p t", p=128))
        bq_sb = singles.tile([128, NP], F32)
        nc.sync.dma_start(out=bq_sb, in_=bq.rearrange("(t p) -> p t", p=128))
        bk_sb = singles.tile([128, NP], F32)
        nc.sync.dma_start(out=bk_sb, in_=bk.rearrange("(t p) -> p t", p=128))
        bv_bc = singles.tile([128, O], F32)
        nc.sync.dma_start(
            out=bv_bc, in_=bass.AP(tensor=bv.tensor, offset=0, ap=[[0, 128], [1, O]])
        )
        # warm the exp table set while the input DMAs stream
        warm_in = singles.tile([128, 1], F32)
        nc.vector.memset(warm_in, 0.0)
        warm_out = singles.tile([128, 1], F32)
        nc.scalar.activation(warm_out, warm_in, EXP)

        # persistent activations. vaug layout per head: col 0 = ones (softmax
        # denominator via the ctx matmul), cols 1:64 = zero pad (PE partition
        # slices must start 32-aligned, so V sits at rows 64:128 of ctx psum),
        # cols 64:128 = V + bias.
        qts = [persist.tile([128, S], BF16, name=f"qt{p}", tag=f"qt{p}") for p in range(NP)]
        kts = [persist.tile([128, S], BF16, name=f"kt{p}", tag=f"kt{p}") for p in range(NP)]
        vaug = [
            persist.tile([128, HL, 128], BF16, name=f"vaug{t}", tag=f"vaug{t}")
            for t in range(ST)
        ]

        # input tiles (kept resident; wk/x first so K/Q proj of pair 0 starts early)
        def load_w(wdram, label):
            wts = []
            for dt in range(8):
                w = work.tile([128, O], BF16, name=f"w{label}{dt}", tag=f"w{label}{dt}")
                nc.sync.dma_start(out=w, in_=wdram[dt * 128 : (dt + 1) * 128, :])
                wts.append(w)
            return wts

        wk_t = load_w(wkt, "k")
        xts = []
        for dt in range(8):
            xti = work.tile([128, S], BF16, name=f"xts{dt}", tag=f"xts{dt}")
            nc.sync.dma_start(out=xti, in_=xt[dt * 128 : (dt + 1) * 128, :])
            xts.append(xti)
        wq_t = load_w(wqt, "q")
        wv_t = load_w(wvt, "v")

        def kq_proj_steps(p):
            """Generator: Q/K projection of pair p, one matmul per yield."""
            for wts, dsts, bias_sb in ((wk_t, kts, bk_sb), (wq_t, qts, bq_sb)):
                for c in range(4):
                    ps = psum.tile([128, 512], F32, name=f"pp{p}_{c}", tag="pp", bufs=2)
                    for dt in range(8):
                        nc.tensor.matmul(
                            ps,
                            wts[dt][:, p * 128 : (p + 1) * 128],
                            xts[dt][:, c * 512 : (c + 1) * 512],
                            start=(dt == 0),
                            stop=(dt == 7),
                        )
                        if dt == 7:
                            nc.vector.tensor_scalar_add(
                                dsts[p][:, c * 512 : (c + 1) * 512],
                                ps,
                                bias_sb[:, p : p + 1],
                            )
                        yield

        # pair-0 Q/K projection up front (not interleaved)
        for _ in kq_proj_steps(0):
            pass

        # V projection + V_aug assembly: vaug[st][:, h, 0] = 1, [:, h, 1:] = V + bv
        for st in range(ST):
            ps = psum.tile([128, O], F32, name=f"ppv{st}", tag="pp", bufs=2)
            for dt in range(8):
                nc.tensor.matmul(
                    ps,
                    xts[dt][:, st * 128 : (st + 1) * 128],
                    wv_t[dt],
                    start=(dt == 0),
                    stop=(dt == 7),
                )
            va = vaug[st]
            nc.vector.memset(va[:, :, 0:1], 1.0)
            nc.vector.memset(va[:, :, 1:DH], 0.0)
            nc.vector.tensor_add(
                va[:, :, DH : 2 * DH],
                ps.rearrange("p (h d) -> p h d", h=HL),
                bv_bc.rearrange("p (h d) -> p h d", h=HL),
            )

        # attention per pair, with pair p+1's Q/K projection interleaved
        for p in range(NP):
            gen = kq_proj_steps(p + 1) if p + 1 < NP else None
            for ic in range(NIC):
                isl = slice(ic * 512, (ic + 1) * 512)
                cxs = [
                    psum.tile([128, 512], F32, name=f"cx{p}_{ic}_{x}", tag="cx", bufs=2)
                    for x in range(2)
                ]
                for jt in range(ST):
                    s = psum.tile(
                        [128, 2, 512], F32, name=f"s{p}_{ic}_{jt}", tag="s", bufs=2
                    )
                    for x in range(2):
                        hp = slice(x * 64, (x + 1) * 64)
                        nc.tensor.matmul(
                            s[:, x, :],
                            kts[p][hp, jt * 128 : (jt + 1) * 128],
                            qts[p][hp, isl],
                            start=True,
                            stop=True,
                            tile_position=(x * 64, 0),
                        )
                    u = work.tile([128, 2, 512], BF16, name=f"u{p}_{ic}_{jt}", tag="u", bufs=12)
                    nc.scalar.activation(
                        u, s, EXP, bias=mask_sb[:, jt : jt + 1], scale=0.125
                    )
                    for x in range(2):
                        nc.tensor.matmul(
                            cxs[x],
                            vaug[jt][:, 2 * p + x, :],
                            u[:, x, :],
                            start=(jt == 0),
                            stop=(jt == ST - 1),
                        )
                    if gen is not None:
                        next(gen, None)
                # drain: row 0 of cxs = softmax denominator, rows 64:128 = ctx
                for x in range(2):
                    rd = work.tile([1, 512], F32, name=f"rd{p}_{ic}_{x}", tag="rd", bufs=2)
                    nc.vector.reciprocal(rd, cxs[x][0:1, :])
                    rdb = work.tile([128, 512], F32, name=f"rdb{p}_{ic}_{x}", tag="rdb", bufs=2)
                    nc.gpsimd.partition_broadcast(rdb, rd, channels=128)
                    ob = work.tile([128, 512], BF16, name=f"ob{p}_{ic}_{x}", tag="ob", bufs=3)
                    nc.vector.tensor_mul(
                        ob[DH:128, :], cxs[x][DH:128, :], rdb[DH:128, :]
                    )
                    hh = 2 * p + x
                    nc.sync.dma_start(
                        out=out[hh * DH : (hh + 1) * DH, isl], in_=ob[DH:128, :]
                    )


def _make_in_maps(hidden_states, attention_mask, Wq, bq, Wk, bk, Wv, bv):
    bf = ml_dtypes.bfloat16
    in_maps = []
    for c in range(8):
        b, hg = divmod(c, 2)
        sl = slice(hg * O, (hg + 1) * O)
        in_maps.append(
            {
                "xt": np.ascontiguousarray(hidden_states[b].T).astype(bf),
                "wqt": np.ascontiguousarray(Wq[sl, :].T).astype(bf),
                "wkt": np.ascontiguousarray(Wk[sl, :].T).astype(bf),
                "wvt": np.ascontiguousarray(Wv[sl, :].T).astype(bf),
                "bq": np.ascontiguousarray(bq[sl]),
                "bk": np.ascontiguousarray(bk[sl]),
                "bv": np.ascontiguousarray(bv[sl]),
                "mask": np.ascontiguousarray(attention_mask[b, 0, 0, :]),
            }
        )
    return in_maps


def _gather(results):
    out = np.empty((B, S, D), dtype=np.float32)
    for c in range(8):
        b, hg = divmod(c, 2)
        out[b, :, hg * O : (hg + 1) * O] = results[c]["out"].astype(np.float32).T
    return out


def kernel(hidden_states, attention_mask, Wq, bq, Wk, bk, Wv, bv, **run_kwargs):
    global _NC_CACHE
    args = [hidden_states, attention_mask, Wq, bq, Wk, bk, Wv, bv]
    args = [np.asarray(a, dtype=np.float32) for a in args]
    if _NC_CACHE is None:
        _NC_CACHE = build_nc()
    in_maps = _make_in_maps(*args)
    res = run_bass_kernel_spmd(_NC_CACHE, in_maps, core_ids=list(range(8)), **run_kwargs)
    kernel.last_result = res
    return _gather(res.results)


# revision 9
# speedup vs baseline: 1.8364x; 1.1131x over previous
"""BERT self-attention (B=4, S=2048, D=1024, H=16) on 8 trn2 NeuronCores.

Sharding: core c -> (batch b = c//2, head-group hg = c%2, 8 heads each).
Each core computes out[b, :, hg*512:(hg+1)*512]; host gathers. Inputs are
pre-transposed AND cast to bf16 on host so matmuls run at full PE rate
(fp32 feed is half rate): xt = X.T [D,S] bf16, w{q,k,v}t = W.T [D,512] bf16.

On-device per core, all matmul operands bf16, PSUM accumulation fp32:
  Q^T, K^T: [o, s] pair-tiles (2 heads / 128 partitions); V_aug [s, h, 65]
  with a leading ones column per head (row 0 of ctx psum = softmax denom).
  Scores S^T[j, i] per head pair packed into one [128, 2, 512] PSUM tile
  (the two heads' K=64 matmuls run concurrently in disjoint PE row groups).
  U = exp(0.125*S^T + mask[j]) in ONE activation instr over 1024 free
  elems -> bf16 SBUF.
  ctx_aug[1+dh, i] accumulated over all 16 j-tiles directly in PSUM.
  Drain: DVE reciprocal of denom row, GpSimd partition-broadcast, DVE
  multiply -> bf16 out tile. Output is stored TRANSPOSED [o, s] in DRAM;
  the host transposes for free during the gather.
  Q/K projections for pair p+1 are interleaved into pair p's attention
  loop (1 matmul per score-tile slot) to fill the ACT-bound PE slack.
"""

import numpy as np
import ml_dtypes

import concourse.bass as bass
import concourse.tile as tile
from concourse import bacc, mybir
from concourse.bass_utils import run_bass_kernel_spmd

B, S, D, H = 4, 2048, 1024, 16
DH = 64
O = 512  # per-core output width (8 heads)
HL = 8  # local heads per core
NP = 4  # head pairs per core
ST = S // 128  # 16 s-tiles
NIC = 4  # i-chunks of 512
F32 = mybir.dt.float32
BF16 = mybir.dt.bfloat16
EXP = mybir.ActivationFunctionType.Exp

_NC_CACHE = None


def build_nc():
    nc = bacc.Bacc(
        "TRN2",
        target_bir_lowering=False,
        debug=False,
        enable_asserts=True,
        num_devices=8,
    )
    xt = nc.dram_tensor("xt", [D, S], BF16, kind="ExternalInput").ap()
    wqt = nc.dram_tensor("wqt", [D, O], BF16, kind="ExternalInput").ap()
    wkt = nc.dram_tensor("wkt", [D, O], BF16, kind="ExternalInput").ap()
    wvt = nc.dram_tensor("wvt", [D, O], BF16, kind="ExternalInput").ap()
    bq = nc.dram_tensor("bq", [O], F32, kind="ExternalInput").ap()
    bk = nc.dram_tensor("bk", [O], F32, kind="ExternalInput").ap()
    bv = nc.dram_tensor("bv", [O], F32, kind="ExternalInput").ap()
    mask = nc.dram_tensor("mask", [S], F32, kind="ExternalInput").ap()
    # transposed output: [o, s]; host transposes during gather
    out = nc.dram_tensor("out", [O, S], BF16, kind="ExternalOutput").ap()

    with tile.TileContext(nc) as tc:
        _emit(nc, tc, xt, wqt, wkt, wvt, bq, bk, bv, mask, out)
    nc.compile()
    return nc


def _emit(nc, tc, xt, wqt, wkt, wvt, bq, bk, bv, mask, out):
    with (
        tc.tile_pool(name="singles", bufs=1) as singles,
        tc.tile_pool(name="persist", bufs=1) as persist,
        tc.tile_pool(name="work", bufs=1) as work,
        tc.tile_pool(name="psum", bufs=1, space="PSUM") as psum,
    ):
        mask_sb = singles.tile([128, ST], F32)
        nc.sync.dma_start(out=mask_sb, in_=mask.rearrange("(t p) -> p t", p=128))
        bq_sb = singles.tile([128, NP], F32)
        nc.sync.dma_start(out=bq_sb, in_=bq.rearrange("(t p) -> p t", p=128))
        bk_sb = singles.tile([128, NP], F32)
        nc.sync.dma_start(out=bk_sb, in_=bk.rearrange("(t p) -> p t", p=128))
        bv_bc = singles.tile([128, O], F32)
        nc.sync.dma_start(
            out=bv_bc, in_=bass.AP(tensor=bv.tensor, offset=0, ap=[[0, 128], [1, O]])
        )
        # warm the exp table set while the input DMAs stream
        warm_in = singles.tile([128, 1], F32)
        nc.vector.memset(warm_in, 0.0)
        warm_out = singles.tile([128, 1], F32)
        nc.scalar.activation(warm_out, warm_in, EXP)

        # persistent activations. vaug layout per head: col 0 = ones (softmax
        # denominator via the ctx matmul), cols 1:64 = zero pad (PE partition
        # slices must start 32-aligned, so V sits at rows 64:128 of ctx psum),
        # cols 64:128 = V + bias.
        qts = [persist.tile([128, S], BF16, name=f"qt{p}", tag=f"qt{p}") for p in range(NP)]
        kts = [persist.tile([128, S], BF16, name=f"kt{p}", tag=f"kt{p}") for p in range(NP)]
        vaug = [
            persist.tile([128, HL, 128], BF16, name=f"vaug{t}", tag=f"vaug{t}")
            for t in range(ST)
        ]

        # input tiles (kept resident; wk/x first so K/Q proj of pair 0 starts early)
        def load_w(wdram, label):
            wts = []
            for dt in range(8):
                w = work.tile([128, O], BF16, name=f"w{label}{dt}", tag=f"w{label}{dt}")
                nc.sync.dma_start(out=w, in_=wdram[dt * 128 : (dt + 1) * 128, :])
                wts.append(w)
            return wts

        wk_t = load_w(wkt, "k")
        wq_t = load_w(wqt, "q")
        # stream the first i-slice of every x tile first so the c=0
        # projection groups (and with them the first scores) start early
        xts = [work.tile([128, S], BF16, name=f"xts{dt}", tag=f"xts{dt}") for dt in range(8)]
        for dt in range(8):
            nc.sync.dma_start(out=xts[dt][:, 0:512], in_=xt[dt * 128 : (dt + 1) * 128, 0:512])
        for dt in range(8):
            nc.sync.dma_start(out=xts[dt][:, 512:S], in_=xt[dt * 128 : (dt + 1) * 128, 512:S])
        wv_t = load_w(wvt, "v")

        def kq_proj_steps(p):
            """Generator: Q/K projection of pair p, one matmul per yield."""
            for c in range(4):
                for wts, dsts, bias_sb in ((wk_t, kts, bk_sb), (wq_t, qts, bq_sb)):
                    ps = psum.tile([128, 512], F32, name=f"pp{p}_{c}", tag="pp", bufs=2)
                    for dt in range(8):
                        nc.tensor.matmul(
                            ps,
                            wts[dt][:, p * 128 : (p + 1) * 128],
                            xts[dt][:, c * 512 : (c + 1) * 512],
                            start=(dt == 0),
                            stop=(dt == 7),
                        )
                        if dt == 7:
                            nc.vector.tensor_scalar_add(
                                dsts[p][:, c * 512 : (c + 1) * 512],
                                ps,
                                bias_sb[:, p : p + 1],
                            )
                        yield

        def vproj_steps():
            """Generator: V projection + V_aug assembly, one s-tile group per
            yield. vaug[st][:, h, 0] = 1, [:, h, 64:128] = V + bv."""
            for st in range(ST):
                ps = psum.tile([128, O], F32, name=f"ppv{st}", tag="pp", bufs=2)
                for dt in range(8):
                    nc.tensor.matmul(
                        ps,
                        xts[dt][:, st * 128 : (st + 1) * 128],
                        wv_t[dt],
                        start=(dt == 0),
                        stop=(dt == 7),
                    )
                va = vaug[st]
                nc.vector.memset(va[:, :, 0:1], 1.0)
                nc.vector.memset(va[:, :, 1:DH], 0.0)
                nc.vector.tensor_add(
                    va[:, :, DH : 2 * DH],
                    ps.rearrange("p (h d) -> p h d", h=HL),
                    bv_bc.rearrange("p (h d) -> p h d", h=HL),
                )
                yield

        # pair-0 Q/K projection up front (not interleaved)
        for _ in kq_proj_steps(0):
            pass
        vgen = vproj_steps()

        # attention per pair, with pair p+1's Q/K projection interleaved
        for p in range(NP):
            gen = kq_proj_steps(p + 1) if p + 1 < NP else None
            for ic in range(NIC):
                isl = slice(ic * 512, (ic + 1) * 512)
                cxs = [
                    psum.tile([128, 512], F32, name=f"cx{p}_{ic}_{x}", tag="cx", bufs=2)
                    for x in range(2)
                ]
                for jt in range(ST):
                    s = psum.tile(
                        [128, 2, 512], F32, name=f"s{p}_{ic}_{jt}", tag="s", bufs=2
                    )
                    for x in range(2):
                        hp = slice(x * 64, (x + 1) * 64)
                        nc.tensor.matmul(
                            s[:, x, :],
                            kts[p][hp, jt * 128 : (jt + 1) * 128],
                            qts[p][hp, isl],
                            start=True,
                            stop=True,
                            tile_position=(x * 64, 0),
                        )
                    u = work.tile([128, 2, 512], BF16, name=f"u{p}_{ic}_{jt}", tag="u", bufs=12)
                    nc.scalar.activation(
                        u, s, EXP, bias=mask_sb[:, jt : jt + 1], scale=0.125
                    )
                    # V projection rides in pair 0's first chunk, one s-tile
                    # group per slot, just ahead of the ctx matmul that needs it
                    next(vgen, None)
                    for x in range(2):
                        nc.tensor.matmul(
                            cxs[x],
                            vaug[jt][:, 2 * p + x, :],
                            u[:, x, :],
                            start=(jt == 0),
                            stop=(jt == ST - 1),
                        )
                    if gen is not None:
                        next(gen, None)
                # drain: row 0 of cxs = softmax denominator, rows 64:128 = ctx.
                # Copy PSUM->SBUF immediately (releases the bank in ~0.4us so
                # the next chunk's ctx accumulation isn't stalled), then
                # normalize off the critical path.
                for x in range(2):
                    st_ = work.tile([128, 512], F32, name=f"st{p}_{ic}_{x}", tag="st", bufs=3)
                    nc.vector.tensor_copy(out=st_, in_=cxs[x])
                    rd = work.tile([1, 512], F32, name=f"rd{p}_{ic}_{x}", tag="rd", bufs=2)
                    nc.vector.reciprocal_approx_fast(out=rd, in_=st_[0:1, :])
                    rdb = work.tile([128, 512], F32, name=f"rdb{p}_{ic}_{x}", tag="rdb", bufs=2)
                    nc.gpsimd.partition_broadcast(rdb, rd, channels=128)
                    ob = work.tile([128, 512], BF16, name=f"ob{p}_{ic}_{x}", tag="ob", bufs=3)
                    nc.vector.tensor_mul(
                        ob[DH:128, :], st_[DH:128, :], rdb[DH:128, :]
                    )
                    hh = 2 * p + x
                    nc.sync.dma_start(
                        out=out[hh * DH : (hh + 1) * DH, isl], in_=ob[DH:128, :]
                    )


def _make_in_maps(hidden_states, attention_mask, Wq, bq, Wk, bk, Wv, bv):
    bf = ml_dtypes.bfloat16
    in_maps = []
    for c in range(8):
        b, hg = divmod(c, 2)
        sl = slice(hg * O, (hg + 1) * O)
        in_maps.append(
            {
                "xt": np.ascontiguousarray(hidden_states[b].T).astype(bf),
                "wqt": np.ascontiguousarray(Wq[sl, :].T).astype(bf),
                "wkt": np.ascontiguousarray(Wk[sl, :].T).astype(bf),
                "wvt": np.ascontiguousarray(Wv[sl, :].T).astype(bf),
                "bq": np.ascontiguousarray(bq[sl]),
                "bk": np.ascontiguousarray(bk[sl]),
                "bv": np.ascontiguousarray(bv[sl]),
                "mask": np.ascontiguousarray(attention_mask[b, 0, 0, :]),
            }
        )
    return in_maps


def _gather(results):
    out = np.empty((B, S, D), dtype=np.float32)
    for c in range(8):
        b, hg = divmod(c, 2)
        out[b, :, hg * O : (hg + 1) * O] = results[c]["out"].astype(np.float32).T
    return out


def kernel(hidden_states, attention_mask, Wq, bq, Wk, bk, Wv, bv, **run_kwargs):
    global _NC_CACHE
    args = [hidden_states, attention_mask, Wq, bq, Wk, bk, Wv, bv]
    args = [np.asarray(a, dtype=np.float32) for a in args]
    if _NC_CACHE is None:
        _NC_CACHE = build_nc()
    in_maps = _make_in_maps(*args)
    res = run_bass_kernel_spmd(_NC_CACHE, in_maps, core_ids=list(range(8)), **run_kwargs)
    kernel.last_result = res
    return _gather(res.results)


# revision 11
# speedup vs baseline: 2.1885x; 1.1918x over previous
"""BERT self-attention (B=4, S=2048, D=1024, H=16) on 8 trn2 NeuronCores.

Sharding: core c -> (batch b = c//2, head-group hg = c%2, 8 heads each).
Each core computes out[b, :, hg*512:(hg+1)*512]; host gathers. Inputs are
pre-transposed AND cast to bf16 on host so matmuls run at full PE rate
(fp32 feed is half rate): xt = X.T [D,S] bf16, w{q,k,v}t = W.T [D,512] bf16.

On-device per core, all matmul operands bf16, PSUM accumulation fp32:
  Q^T, K^T: [o, s] pair-tiles (2 heads / 128 partitions); V_aug [s, h, 65]
  with a leading ones column per head (row 0 of ctx psum = softmax denom).
  Scores S^T[j, i] per head pair packed into one [128, 2, 512] PSUM tile
  (the two heads' K=64 matmuls run concurrently in disjoint PE row groups).
  U = exp(0.125*S^T + mask[j]) in ONE activation instr over 1024 free
  elems -> bf16 SBUF.
  ctx_aug[1+dh, i] accumulated over all 16 j-tiles directly in PSUM.
  Drain: DVE reciprocal of denom row, GpSimd partition-broadcast, DVE
  multiply -> bf16 out tile. Output is stored TRANSPOSED [o, s] in DRAM;
  the host transposes for free during the gather.
  Q/K projections for pair p+1 are interleaved into pair p's attention
  loop (1 matmul per score-tile slot) to fill the ACT-bound PE slack.
"""

import numpy as np
import ml_dtypes

import concourse.bass as bass
import concourse.tile as tile
from concourse import bacc, mybir
from concourse.bass_utils import run_bass_kernel_spmd

B, S, D, H = 4, 2048, 1024, 16
DH = 64
O = 512  # per-core output width (8 heads)
HL = 8  # local heads per core
NP = 4  # head pairs per core
ST = S // 128  # 16 s-tiles
NIC = 4  # i-chunks of 512
F32 = mybir.dt.float32
BF16 = mybir.dt.bfloat16
EXP = mybir.ActivationFunctionType.Exp

_NC_CACHE = None


def build_nc():
    nc = bacc.Bacc(
        "TRN2",
        target_bir_lowering=False,
        debug=False,
        enable_asserts=True,
        num_devices=8,
    )
    xt = nc.dram_tensor("xt", [D, S], BF16, kind="ExternalInput").ap()
    wqt = nc.dram_tensor("wqt", [D, O], BF16, kind="ExternalInput").ap()
    wkt = nc.dram_tensor("wkt", [D, O], BF16, kind="ExternalInput").ap()
    wvt = nc.dram_tensor("wvt", [D, O], BF16, kind="ExternalInput").ap()
    bq = nc.dram_tensor("bq", [O], F32, kind="ExternalInput").ap()
    bk = nc.dram_tensor("bk", [O], F32, kind="ExternalInput").ap()
    bv = nc.dram_tensor("bv", [O], F32, kind="ExternalInput").ap()
    mask = nc.dram_tensor("mask", [S], F32, kind="ExternalInput").ap()
    # transposed output: [o, s]; host transposes during gather
    out = nc.dram_tensor("out", [O, S], BF16, kind="ExternalOutput").ap()

    with tile.TileContext(nc) as tc:
        _emit(nc, tc, xt, wqt, wkt, wvt, bq, bk, bv, mask, out)
    nc.compile()
    return nc


def _emit(nc, tc, xt, wqt, wkt, wvt, bq, bk, bv, mask, out):
    with (
        tc.tile_pool(name="singles", bufs=1) as singles,
        tc.tile_pool(name="persist", bufs=1) as persist,
        tc.tile_pool(name="work", bufs=1) as work,
        tc.tile_pool(name="psum", bufs=1, space="PSUM") as psum,
    ):
        mask_sb = singles.tile([128, ST], F32)
        nc.sync.dma_start(out=mask_sb, in_=mask.rearrange("(t p) -> p t", p=128))
        bq_sb = singles.tile([128, NP], F32)
        nc.sync.dma_start(out=bq_sb, in_=bq.rearrange("(t p) -> p t", p=128))
        bk_sb = singles.tile([128, NP], F32)
        nc.sync.dma_start(out=bk_sb, in_=bk.rearrange("(t p) -> p t", p=128))
        bv_bc = singles.tile([128, O], F32)
        nc.sync.dma_start(
            out=bv_bc, in_=bass.AP(tensor=bv.tensor, offset=0, ap=[[0, 128], [1, O]])
        )
        # warm the exp table set while the input DMAs stream
        warm_in = singles.tile([128, 1], F32)
        nc.vector.memset(warm_in, 0.0)
        warm_out = singles.tile([128, 1], F32)
        nc.scalar.activation(warm_out, warm_in, EXP)

        # persistent activations. vaug layout per head: col 0 = ones (softmax
        # denominator via the ctx matmul), cols 1:64 = zero pad (PE partition
        # slices must start 32-aligned, so V sits at rows 64:128 of ctx psum),
        # cols 64:128 = V + bias.
        qts = [persist.tile([128, S], BF16, name=f"qt{p}", tag=f"qt{p}") for p in range(NP)]
        kts = [persist.tile([128, S], BF16, name=f"kt{p}", tag=f"kt{p}") for p in range(NP)]
        vaug = [
            persist.tile([128, HL, 128], BF16, name=f"vaug{t}", tag=f"vaug{t}")
            for t in range(ST)
        ]

        # input tiles (kept resident; wk/x first so K/Q proj of pair 0 starts early)
        def load_w(wdram, label):
            wts = []
            for dt in range(8):
                w = work.tile([128, O], BF16, name=f"w{label}{dt}", tag=f"w{label}{dt}")
                nc.sync.dma_start(out=w, in_=wdram[dt * 128 : (dt + 1) * 128, :])
                wts.append(w)
            return wts

        # DMA order: wk, then the first i-slice of x (k c=0 group can start),
        # then wq (q c=0), then the rest of x, then wv
        wk_t = load_w(wkt, "k")
        xts = [work.tile([128, S], BF16, name=f"xts{dt}", tag=f"xts{dt}") for dt in range(8)]
        for dt in range(8):
            nc.sync.dma_start(out=xts[dt][:, 0:512], in_=xt[dt * 128 : (dt + 1) * 128, 0:512])
        wq_t = load_w(wqt, "q")
        for dt in range(8):
            nc.sync.dma_start(out=xts[dt][:, 512:S], in_=xt[dt * 128 : (dt + 1) * 128, 512:S])
        wv_t = load_w(wvt, "v")

        def kq_proj_steps(p):
            """Generator: Q/K projection of pair p, one matmul per yield."""
            for c in range(4):
                for wts, dsts, bias_sb in ((wk_t, kts, bk_sb), (wq_t, qts, bq_sb)):
                    ps = psum.tile([128, 512], F32, name=f"pp{p}_{c}", tag="pp", bufs=2)
                    for dt in range(8):
                        nc.tensor.matmul(
                            ps,
                            wts[dt][:, p * 128 : (p + 1) * 128],
                            xts[dt][:, c * 512 : (c + 1) * 512],
                            start=(dt == 0),
                            stop=(dt == 7),
                        )
                        if dt == 7:
                            nc.vector.tensor_scalar_add(
                                dsts[p][:, c * 512 : (c + 1) * 512],
                                ps,
                                bias_sb[:, p : p + 1],
                            )
                        yield

        def vproj_steps():
            """Generator: V projection + V_aug assembly, one s-tile group per
            yield. vaug[st][:, h, 0] = 1, [:, h, 64:128] = V + bv."""
            for st in range(ST):
                ps = psum.tile([128, O], F32, name=f"ppv{st}", tag="pp", bufs=2)
                for dt in range(8):
                    nc.tensor.matmul(
                        ps,
                        xts[dt][:, st * 128 : (st + 1) * 128],
                        wv_t[dt],
                        start=(dt == 0),
                        stop=(dt == 7),
                    )
                va = vaug[st]
                nc.vector.memset(va[:, :, 0:1], 1.0)
                nc.vector.memset(va[:, :, 1:DH], 0.0)
                nc.vector.tensor_add(
                    va[:, :, DH : 2 * DH],
                    ps.rearrange("p (h d) -> p h d", h=HL),
                    bv_bc.rearrange("p (h d) -> p h d", h=HL),
                )
                yield

        # pair-0 Q/K projection up front (not interleaved)
        for _ in kq_proj_steps(0):
            pass
        vgen = vproj_steps()

        # attention per pair, with pair p+1's Q/K projection interleaved
        for p in range(NP):
            gen = kq_proj_steps(p + 1) if p + 1 < NP else None
            for ic in range(NIC):
                isl = slice(ic * 512, (ic + 1) * 512)
                cxs = [
                    psum.tile([128, 512], F32, name=f"cx{p}_{ic}_{x}", tag="cx", bufs=2)
                    for x in range(2)
                ]
                def emit_ctx(jt, u):
                    for x in range(2):
                        nc.tensor.matmul(
                            cxs[x],
                            vaug[jt][:, 2 * p + x, :],
                            u[:, x, :],
                            start=(jt == 0),
                            stop=(jt == ST - 1),
                        )

                prev = None
                for jt in range(ST):
                    s = psum.tile(
                        [128, 2, 512], F32, name=f"s{p}_{ic}_{jt}", tag="s", bufs=2
                    )
                    for x in range(2):
                        hp = slice(x * 64, (x + 1) * 64)
                        nc.tensor.matmul(
                            s[:, x, :],
                            kts[p][hp, jt * 128 : (jt + 1) * 128],
                            qts[p][hp, isl],
                            start=True,
                            stop=True,
                            tile_position=(x * 64, 0),
                        )
                    u = work.tile([128, 2, 512], BF16, name=f"u{p}_{ic}_{jt}", tag="u", bufs=12)
                    nc.scalar.activation(
                        u.rearrange("p x i -> p (x i)"),
                        s.rearrange("p x i -> p (x i)"),
                        EXP,
                        bias=mask_sb[:, jt : jt + 1],
                        scale=0.125,
                    )
                    # V projection rides in pair 0's first chunk, one s-tile
                    # group per slot, just ahead of the ctx matmul that needs it
                    next(vgen, None)
                    # always-ready projection work fills PE time while exp runs
                    if gen is not None:
                        next(gen, None)
                    # ctx one slot behind: its exp has already finished, so the
                    # PE queue never blocks a full exp latency mid-slot
                    if prev is not None:
                        emit_ctx(*prev)
                    prev = (jt, u)
                emit_ctx(*prev)
                # drain: row 0 of cxs = softmax denominator, rows 64:128 = ctx.
                # Copy PSUM->SBUF immediately (releases the bank in ~0.4us so
                # the next chunk's ctx accumulation isn't stalled), then
                # normalize off the critical path.
                for x in range(2):
                    st_ = work.tile([128, 512], F32, name=f"st{p}_{ic}_{x}", tag="st", bufs=3)
                    nc.vector.tensor_copy(out=st_, in_=cxs[x])
                    rd = work.tile([1, 512], F32, name=f"rd{p}_{ic}_{x}", tag="rd", bufs=2)
                    nc.vector.reciprocal_approx_fast(out=rd, in_=st_[0:1, :])
                    rdb = work.tile([128, 512], F32, name=f"rdb{p}_{ic}_{x}", tag="rdb", bufs=2)
                    nc.gpsimd.partition_broadcast(rdb, rd, channels=128)
                    ob = work.tile([128, 512], BF16, name=f"ob{p}_{ic}_{x}", tag="ob", bufs=3)
                    nc.vector.tensor_mul(
                        ob[DH:128, :], st_[DH:128, :], rdb[DH:128, :]
                    )
                    hh = 2 * p + x
                    nc.sync.dma_start(
                        out=out[hh * DH : (hh + 1) * DH, isl], in_=ob[DH:128, :]
                    )


def _make_in_maps(hidden_states, attention_mask, Wq, bq, Wk, bk, Wv, bv):
    bf = ml_dtypes.bfloat16
    in_maps = []
    for c in range(8):
        b, hg = divmod(c, 2)
        sl = slice(hg * O, (hg + 1) * O)
        in_maps.append(
            {
                "xt": np.ascontiguousarray(hidden_states[b].T).astype(bf),
                "wqt": np.ascontiguousarray(Wq[sl, :].T).astype(bf),
                "wkt": np.ascontiguousarray(Wk[sl, :].T).astype(bf),
                "wvt": np.ascontiguousarray(Wv[sl, :].T).astype(bf),
                "bq": np.ascontiguousarray(bq[sl]),
                "bk": np.ascontiguousarray(bk[sl]),
                "bv": np.ascontiguousarray(bv[sl]),
                "mask": np.ascontiguousarray(attention_mask[b, 0, 0, :]),
            }
        )
    return in_maps


def _gather(results):
    out = np.empty((B, S, D), dtype=np.float32)
    for c in range(8):
        b, hg = divmod(c, 2)
        out[b, :, hg * O : (hg + 1) * O] = results[c]["out"].astype(np.float32).T
    return out


def kernel(hidden_states, attention_mask, Wq, bq, Wk, bk, Wv, bv, **run_kwargs):
    global _NC_CACHE
    args = [hidden_states, attention_mask, Wq, bq, Wk, bk, Wv, bv]
    args = [np.asarray(a, dtype=np.float32) for a in args]
    if _NC_CACHE is None:
        _NC_CACHE = build_nc()
    in_maps = _make_in_maps(*args)
    res = run_bass_kernel_spmd(_NC_CACHE, in_maps, core_ids=list(range(8)), **run_kwargs)
    kernel.last_result = res
    return _gather(res.results)
